# revision 1
# baseline (speedup 1.0000x reference)
"""Bundle-adjustment loss kernel for 8 Trainium2 NeuronCores.

Data-parallel over the image axis M: each core processes MC=12800 images
(zero-padded from 100000/8=12500; the len-loss contribution of padded
images is corrected analytically on the host).

Device layout: partition dim = (camera, point) = 96, free dim = images.
Camera intrinsics/distortion become per-partition scalars, which lets the
whole distortion polynomial run as a handful of fused DVE instructions.
PE does R@X+t as fp16 matmuls W[10,96].T @ XT[10,512] -> PSUM fp32.
"""

import numpy as np

M_TOTAL = 100000
C = 32
NCORES = 8
CHUNK = 512
MC = 12800           # images per core (25 chunks of 512)
NCH = MC // CHUNK    # 25
CP = 96              # (camera, point) pairs
W_LOSS = 0.01        # LINE_W = LEN_W = REPROJ_W

_REGISTERED = {}
_NC_CACHE = {}


def _apply_tile_patch():
    """This walrus build rejects Tile's kernel-tail drain carrying every
    semaphore wait on one instruction ("Too many sync wait commands").
    Emit one wait_ge per live semaphore instead."""
    from concourse import tile

    if getattr(tile.TileContext, "_ba_drain_patched", False):
        return

    def _drain_and_barrier(self, tick_clock, wait_clock):
        nc = self.nc
        ticks = list(tick_clock.global_clock)
        allocated = wait_clock.sems.allocated()
        for key, sem in allocated.items():
            t = ticks[int(key)]
            if t > 0:
                nc.sync.wait_ge(sem, t)
        nc.sync.drain()
        nc.all_engine_barrier()
        assert self.sems is not None
        popped = nc._tile_sem_poison_stack.pop()
        assert popped is self._sem_poison
        nc.clear_and_free_semaphores(list(self.sems.allocated().values()))
        nc.all_engine_barrier()

    tile.TileContext._drain_and_barrier = _drain_and_barrier
    tile.TileContext._ba_drain_patched = True


def _spill_excess_waits(nc, cap=1):
    """This walrus build's ISA structs accept very few sync-wait slots
    per compute instruction. Spill waits beyond `cap` onto InstNoOp
    carriers inserted just before the instruction on the same engine."""
    import concourse.mybir as mybir
    import bass_rust

    fragile = {
        "InstTensorScalarPtr", "InstActivation", "InstReciprocal",
        "InstTensorReduce", "InstMatmult", "InstTensorCopy",
        "InstTensorTensor", "InstLdweights", "InstMemset", "InstIota",
        "InstTensorTensorReduce", "InstPool", "InstDMACopy", "InstDMA",
        "InstDmaTransposeAnt",
    }
    n_nop = 0
    for bb in nc.m.functions[0].blocks:
        il = bb.instructions
        out_list = []
        for inst in il:
            si = inst.sync_info
            if (si is not None and type(inst).__name__ in fragile
                    and len(si.on_wait) > cap):
                waits = list(si.on_wait)
                keep, spill = waits[:cap], waits[cap:]
                for wv in spill:
                    nop = mybir.InstNoOp(name=f"ba_waitnop_{n_nop}")
                    n_nop += 1
                    nop.engine = inst.engine
                    nop.sync_info = bass_rust.SyncInfo(
                        on_wait=[wv], on_update=[])
                    out_list.append(nop)
                inst.sync_info = bass_rust.SyncInfo(
                    on_wait=keep, on_update=list(si.on_update))
            out_list.append(inst)
        if len(out_list) != len(il):
            bb.instructions = out_list
    return n_nop


def _register_custom_ops():
    """Register our fused DVE ops in dve_ops.OPS (process-local)."""
    if _REGISTERED:
        return _REGISTERED
    from concourse import dve_ops as dvo
    from concourse.dve_spec import (
        Spec, Src0, Src1, C0, C1, C3, One, sq, lower,
        _has_src1 as has_src1, _spill_c3_to_src1,
    )
    from concourse.dve_uop import DveOpSpec

    def mk(name, body, reference, accum=None):
        if name in dvo._SUB_OPCODE_FOR_NAME:
            return next(o for o in dvo.OPS if o.name == name)
        spec = Spec(body=body, accum=accum, reference=reference)
        row = len(dvo.OPS)
        assert row < 0x20, "DVE opcode rows exhausted"
        shas = {}
        for ver in ("v3", "v4"):
            try:
                tmp = DveOpSpec(name=name, opcode=row,
                                uops=lower(spec, ver=ver),
                                rd1_en=has_src1(spec))
                shas[ver] = tmp.sha(ver)
            except Exception:
                pass
        op = dvo.DveOp(name, spec, subdim=False, uops_sha=shas)
        dvo.OPS.append(op)
        dvo._SUB_OPCODE_FOR_NAME[name] = row
        return op

    # r2 = x0n^2 + x1n^2
    _REGISTERED["R2"] = mk(
        "BA_R2", sq(Src0) + sq(Src1),
        lambda in0, in1, s0, s1, imm2: in0 * in0 + in1 * in1)
    # radial = ((k3*r2 + k2)*r2 + k1)*r2 + 1 ; k3=s0, k2=s1, k1 via in1 latch
    _REGISTERED["HORNER"] = mk(
        "BA_HORNER",
        _spill_c3_to_src1(((C0 * Src0 + C1) * Src0 + C3) * Src0 + One),
        lambda in0, in1, s0, s1, imm2:
            ((s0 * in0 + s1) * in0 + in1[..., :1]) * in0 + 1.0)
    # obsadj = S0 - s0*S1 - s1   (S0=obs, S1=r2, s0=f*dist, s1=principal)
    _REGISTERED["OBSR"] = mk(
        "BA_OBSR", Src0 - (C0 * Src1 + C1),
        lambda in0, in1, s0, s1, imm2: in0 - (s0 * in1 + s1))
    # d2 = (S0 - s0*S1)^2   (S0=obsadj, S1=x_n*(radial+w), s0=f)
    _REGISTERED["SQDS"] = mk(
        "BA_SQDS", sq(Src0 - C0 * Src1),
        lambda in0, in1, s0, s1, imm2: (in0 - s0 * in1) ** 2)
    # g = s0*S0 + s1*S1  (line expectation)
    _REGISTERED["AXBY"] = mk(
        "BA_AXBY", C0 * Src0 + C1 * Src1,
        lambda in0, in1, s0, s1, imm2: s0 * in0 + s1 * in1)
    # sqdiff = (S0 - S1)^2
    _REGISTERED["SQDIFF"] = mk(
        "BA_SQDIFF", sq(Src0 - Src1),
        lambda in0, in1, s0, s1, imm2: (in0 - in1) ** 2)
    return _REGISTERED


def _build_nc(a_coef, b_coef, s_len, reps=1, variant="full"):
    """Build the SPMD Bass module (same program on all 8 cores)."""
    key = (a_coef, b_coef, s_len, reps, variant)
    if key in _NC_CACHE:
        return _NC_CACHE[key]
    import concourse.bass as bass
    import concourse.mybir as mybir
    from concourse import tile

    _apply_tile_patch()
    F32 = mybir.dt.float32
    F16 = mybir.dt.float16
    ALU = mybir.AluOpType
    ACT = mybir.ActivationFunctionType

    nc = bass.Bass(trn_type="TRN2")
    # inputs
    obs_u = nc.declare_dram_parameter("obs_u", [CP, MC], F16, isOutput=False)
    obs_v = nc.declare_dram_parameter("obs_v", [CP, MC], F16, isOutput=False)
    maskf = nc.declare_dram_parameter("maskf", [CP, MC], F16, isOutput=False)
    xt = nc.declare_dram_parameter("xt", [10, MC], F16, isOutput=False)
    xn = nc.declare_dram_parameter("xn", [MC, 9], F32, isOutput=False)
    wmat = nc.declare_dram_parameter("wmat", [3, 10, CP], F16, isOutput=False)
    # camera constants as materialized [CP, CHUNK] broadcast planes
    # (this walrus build rejects per-partition scalar APs / bcast APs):
    # 0:k1 1:k2 2:k3 3:2p1 4:2p2 5:fx*p2 6:fy*p1 7:fx 8:fy
    consts = nc.declare_dram_parameter("consts", [9, CP, CHUNK], F16,
                                       isOutput=False)
    out = nc.declare_dram_parameter("out", [224], F32, isOutput=True)

    with tile.TileContext(nc) as tc:
        with (
            tc.tile_pool(name="const", bufs=1) as cpool,
            tc.tile_pool(name="io", bufs=4) as iop,
            tc.tile_pool(name="work", bufs=3) as wk,
            tc.tile_pool(name="stage", bufs=1) as stg,
            tc.tile_pool(name="psum", bufs=2, space=bass.MemorySpace.PSUM) as pp,
        ):
            # --- constants ---
            cts = []
            for i in range(9):
                ct = cpool.tile([CP, CHUNK], F16, tag=f"ct{i}", name=f"ct{i}")
                nc.sync.dma_start(ct[:], consts[i])
                cts.append(ct)
            (K1T, K2T, K3T, TP1T, TP2T, FXP2T, FYP1T, FXT, FYT) = cts
            w_ts = []
            for i in range(3):
                w_ti = cpool.tile([10, CP], F16, tag=f"wm{i}", name=f"wm{i}")
                nc.sync.dma_start(w_ti[:], wmat[i])
                w_ts.append(w_ti)

            # staging accumulators (written col-ranges per chunk)
            pt_stage = stg.tile([CP, NCH], F32, tag="pts")
            line2_st = stg.tile([128, 4 * NCH], F32, tag="l2s")
            len2_st = stg.tile([128, 4 * NCH], F32, tag="n2s")

            for ci_rep in range(NCH * reps):
                ci = ci_rep % NCH
                cs = ci * CHUNK
                # ---- loads ----
                ou = iop.tile([CP, CHUNK], F16, tag="ou")
                ov = iop.tile([CP, CHUNK], F16, tag="ov")
                mk_t = iop.tile([CP, CHUNK], F16, tag="mk")
                xt_t = iop.tile([10, CHUNK], F16, tag="xt")
                xn_t = iop.tile([128, 36], F32, tag="xn")
                nc.sync.dma_start(ou[:], obs_u[:, cs:cs + CHUNK])
                nc.sync.dma_start(ov[:], obs_v[:, cs:cs + CHUNK])
                nc.sync.dma_start(mk_t[:], maskf[:, cs:cs + CHUNK])
                nc.sync.dma_start(xt_t[:], xt[:, cs:cs + CHUNK])
                nc.sync.dma_start(
                    xn_t[:], xn.rearrange("(c p a) j -> c p (a j)",
                                          c=NCH, p=128)[ci])

                if variant == "dmaonly":
                    nc.vector.tensor_reduce(
                        pt_stage[:, ci:ci + 1], ou[:],
                        mybir.AxisListType.X, ALU.add)
                    nc.vector.tensor_reduce(
                        line2_st[:, 4 * ci:4 * ci + 4],
                        xn_t[:].rearrange("p (a j) -> p a j", a=4)[:, :, 0:3],
                        mybir.AxisListType.X, ALU.add)
                    nc.vector.tensor_reduce(
                        len2_st[:, 4 * ci:4 * ci + 4],
                        xn_t[:].rearrange("p (a j) -> p a j", a=4)[:, :, 3:6],
                        mybir.AxisListType.X, ALU.add)
                    continue
                # ---- PE: x_i[cp, m] = W_i.T @ XT ----
                px = [pp.tile([CP, CHUNK], F32, tag=f"px{i}",
                              name=f"px{i}") for i in range(3)]
                for i in range(3):
                    nc.tensor.matmul(px[i][:], w_ts[i][:], xt_t[:])

                # ---- normalized coords ----
                def wt(tag, dt=F32):
                    return wk.tile([CP, CHUNK], dt, tag=tag, name=tag)

                iz = wt("iz")
                if variant in ("norecip", "skeleton"):
                    nc.vector.tensor_copy(iz[:], px[2][:])
                else:
                    nc.vector.reciprocal(iz[:], px[2][:])
                x0n = wt("x0n", F16)
                x1n = wt("x1n", F16)
                nc.vector.scalar_tensor_tensor(
                    x0n[:], px[0][:], 1.0, iz[:], ALU.mult, ALU.mult)
                nc.vector.scalar_tensor_tensor(
                    x1n[:], px[1][:], 1.0, iz[:], ALU.mult, ALU.mult)
                u_sq = wt("u_sq", F16); w_sq = wt("w_sq", F16)
                if variant in ("noact", "skeleton"):
                    nc.vector.scalar_tensor_tensor(
                        u_sq[:], x0n[:], 1.0, x0n[:], ALU.mult, ALU.mult)
                    nc.vector.scalar_tensor_tensor(
                        w_sq[:], x1n[:], 1.0, x1n[:], ALU.mult, ALU.mult)
                else:
                    nc.scalar.activation(u_sq[:], x0n[:], ACT.Square)
                    nc.scalar.activation(w_sq[:], x1n[:], ACT.Square)
                r2 = wt("r2", F16)
                nc.vector.scalar_tensor_tensor(
                    r2[:], u_sq[:], 1.0, w_sq[:], ALU.mult, ALU.add)

                # ---- distortion: radial-1 = ((k3*r2+k2)*r2+k1)*r2 ----
                h1 = wt("h1", F16); h2 = wt("h2", F16); h3 = wt("h3", F16)
                nc.vector.scalar_tensor_tensor(
                    h1[:], r2[:], 1.0, K3T[:], ALU.mult, ALU.mult)
                nc.vector.scalar_tensor_tensor(
                    h2[:], h1[:], 1.0, K2T[:], ALU.mult, ALU.add)
                nc.vector.scalar_tensor_tensor(
                    h3[:], h2[:], 1.0, r2[:], ALU.mult, ALU.mult)
                h4 = wt("h4", F16); h5 = wt("h5", F16)
                nc.vector.scalar_tensor_tensor(
                    h4[:], h3[:], 1.0, K1T[:], ALU.mult, ALU.add)
                nc.vector.scalar_tensor_tensor(
                    h5[:], h4[:], 1.0, r2[:], ALU.mult, ALU.mult)
                # w = 2p1*x1n + 2p2*x0n (shared); ra = radial + w
                a1 = wt("a1", F16); a2 = wt("a2", F16); s12 = wt("s12", F16)
                nc.vector.scalar_tensor_tensor(
                    a1[:], x1n[:], 1.0, TP1T[:], ALU.mult, ALU.mult)
                nc.vector.scalar_tensor_tensor(
                    a2[:], x0n[:], 1.0, TP2T[:], ALU.mult, ALU.mult)
                nc.vector.scalar_tensor_tensor(
                    s12[:], a1[:], 1.0, a2[:], ALU.add, ALU.add)
                ra = wt("ra", F16)
                nc.vector.scalar_tensor_tensor(
                    ra[:], h5[:], 1.0, s12[:], ALU.mult, ALU.add)
                mu = wt("mu", F16); mv = wt("mv", F16)
                nc.vector.scalar_tensor_tensor(
                    mu[:], x0n[:], 1.0, ra[:], ALU.mult, ALU.mult)
                nc.vector.scalar_tensor_tensor(
                    mv[:], x1n[:], 1.0, ra[:], ALU.mult, ALU.mult)

                # ---- pixel residuals (obs pre-folded by -u0/-v0 on host) ---
                # du = (obs_u' - fx*p2*r2) - fx*mu   (sign flip is harmless)
                nu = wt("nu", F16); du0 = wt("du0"); fxmu = wt("fxmu")
                du = wt("du")
                nc.vector.scalar_tensor_tensor(
                    nu[:], r2[:], 1.0, FXP2T[:], ALU.mult, ALU.mult)
                nc.vector.scalar_tensor_tensor(
                    du0[:], nu[:], -1.0, ou[:], ALU.mult, ALU.add)
                nc.vector.scalar_tensor_tensor(
                    fxmu[:], mu[:], 1.0, FXT[:], ALU.mult, ALU.mult)
                nc.vector.scalar_tensor_tensor(
                    du[:], fxmu[:], -1.0, du0[:], ALU.mult, ALU.add)
                nv = wt("nv", F16); dv0 = wt("dv0"); fymv = wt("fymv")
                dv = wt("dv")
                nc.vector.scalar_tensor_tensor(
                    nv[:], r2[:], 1.0, FYP1T[:], ALU.mult, ALU.mult)
                nc.vector.scalar_tensor_tensor(
                    dv0[:], nv[:], -1.0, ov[:], ALU.mult, ALU.add)
                nc.vector.scalar_tensor_tensor(
                    fymv[:], mv[:], 1.0, FYT[:], ALU.mult, ALU.mult)
                nc.vector.scalar_tensor_tensor(
                    dv[:], fymv[:], -1.0, dv0[:], ALU.mult, ALU.add)
                d2u = wt("d2u"); d2v = wt("d2v")
                if variant in ("noact", "skeleton"):
                    nc.vector.scalar_tensor_tensor(
                        d2u[:], du[:], 1.0, du[:], ALU.mult, ALU.mult)
                    nc.vector.scalar_tensor_tensor(
                        d2v[:], dv[:], 1.0, dv[:], ALU.mult, ALU.mult)
                else:
                    nc.scalar.activation(d2u[:], du[:], ACT.Square)
                    nc.scalar.activation(d2v[:], dv[:], ACT.Square)
                e = wt("e"); em = wt("em")
                nc.vector.scalar_tensor_tensor(
                    e[:], d2u[:], 1.0, d2v[:], ALU.mult, ALU.add)
                nc.vector.scalar_tensor_tensor(
                    em[:], e[:], 1.0, mk_t[:], ALU.mult, ALU.mult)
                if variant in ("noact", "skeleton"):
                    nc.vector.tensor_reduce(
                        pt_stage[:, ci:ci + 1], em[:],
                        mybir.AxisListType.X, ALU.add)
                else:
                    junk = wk.tile([CP, CHUNK], F32, tag="junk")
                    nc.scalar.activation(junk[:], em[:], ACT.Sqrt,
                                         accum_out=pt_stage[:, ci:ci + 1])

                # ---- line/len losses (m-partition side pipeline) ----
                x0s = xn_t[:].rearrange("p (a j) -> p a j", a=4)[:, :, 0:3]
                x1s = xn_t[:].rearrange("p (a j) -> p a j", a=4)[:, :, 3:6]
                x2s = xn_t[:].rearrange("p (a j) -> p a j", a=4)[:, :, 6:9]
                t_b = wk.tile([128, 4, 3], F32, tag="t_b")
                g = wk.tile([128, 4, 3], F32, tag="g")
                nc.vector.tensor_scalar_mul(t_b[:], x2s, b_coef)
                nc.vector.scalar_tensor_tensor(
                    g[:], x0s, a_coef, t_b[:], ALU.mult, ALU.add)
                dlt = wk.tile([128, 4, 3], F32, tag="dlt")
                nc.vector.scalar_tensor_tensor(
                    dlt[:], g[:], -1.0, x1s, ALU.mult, ALU.add)
                dl = wk.tile([128, 4, 3], F32, tag="dl")
                nc.vector.scalar_tensor_tensor(
                    dl[:], dlt[:], 1.0, dlt[:], ALU.mult, ALU.mult)
                dnt = wk.tile([128, 4, 3], F32, tag="dnt")
                nc.vector.scalar_tensor_tensor(
                    dnt[:], x2s, -1.0, x0s, ALU.mult, ALU.add)
                dn = wk.tile([128, 4, 3], F32, tag="dn")
                nc.vector.scalar_tensor_tensor(
                    dn[:], dnt[:], 1.0, dnt[:], ALU.mult, ALU.mult)
                nc.vector.tensor_reduce(
                    line2_st[:, 4 * ci:4 * ci + 4], dl[:],
                    mybir.AxisListType.X, ALU.add)
                nc.vector.tensor_reduce(
                    len2_st[:, 4 * ci:4 * ci + 4], dn[:],
                    mybir.AxisListType.X, ALU.add)

            # ---- epilogue ----
            lsq = stg.tile([128, 4 * NCH], F32, tag="lsq")
            nc.scalar.activation(lsq[:], line2_st[:], ACT.Sqrt)
            lenq = stg.tile([128, 4 * NCH], F32, tag="lenq")
            nc.scalar.activation(lenq[:], len2_st[:], ACT.Sqrt)
            lena = stg.tile([128, 4 * NCH], F32, tag="lena")
            negs = cpool.tile([128, 1], F32, tag="negs")
            nc.gpsimd.memset(negs[:], -s_len)
            nc.scalar.activation(lena[:], lenq[:], ACT.Abs, bias=negs[:])
            comb = stg.tile([128, 4 * NCH], F32, tag="comb")
            nc.vector.scalar_tensor_tensor(
                comb[:], lsq[:], 1.0, lena[:], ALU.mult, ALU.add)
            llred = stg.tile([128, 1], F32, tag="llred")
            nc.vector.tensor_reduce(llred[:], comb[:],
                                    mybir.AxisListType.X, ALU.add)
            ptred = stg.tile([CP, 1], F32, tag="ptred")
            nc.vector.tensor_reduce(ptred[:], pt_stage[:],
                                    mybir.AxisListType.X, ALU.add)
            nc.sync.dma_start(out[0:CP], ptred[:])
            nc.sync.dma_start(out[CP:224], llred[:])

    _spill_excess_waits(nc)
    _NC_CACHE[key] = nc
    return nc


def kernel(pole, pole_3ds, pole_2ds, mask, K, dist, R, t):
    pole = np.asarray(pole, np.float32)
    pole_3ds = np.asarray(pole_3ds, np.float32)
    pole_2ds = np.asarray(pole_2ds, np.float32)
    mask = np.asarray(mask)
    K = np.asarray(K, np.float32)
    dist = np.asarray(dist, np.float32)
    R = np.asarray(R, np.float32)
    t = np.asarray(t, np.float32)

    s = float(pole[0] + pole[1])
    a_coef = float(pole[1] / s)   # coefficient of X0 in exp_p1
    b_coef = float(pole[0] / s)   # coefficient of X2

    # ---- camera-constant planes: rows are (c, p) = c*3+p ----
    def rep(v):  # [C] -> [CP]
        return np.repeat(v.astype(np.float32), 3)
    cvecs = [rep(dist[:, 0]), rep(dist[:, 1]), rep(dist[:, 4]),
             rep(2.0 * dist[:, 2]), rep(2.0 * dist[:, 3]),
             rep(K[:, 0, 0] * dist[:, 3]), rep(K[:, 1, 1] * dist[:, 2]),
             rep(K[:, 0, 0]), rep(K[:, 1, 1])]
    consts = np.ascontiguousarray(
        np.broadcast_to(np.stack(cvecs)[:, :, None], (9, CP, CHUNK)),
        dtype=np.float16)
    u0_cp = rep(K[:, 0, 2])   # [CP]
    v0_cp = rep(K[:, 1, 2])

    # ---- matmul weights: W[i, (p,j), c*3+p] = R[c,i,j]; row 9 = t[c,i] ----
    wmat = np.zeros((3, 10, CP), np.float32)
    for p in range(3):
        # rows p*3+j, cols c*3+p
        wmat[:, p * 3:p * 3 + 3, p::3] = R.transpose(1, 2, 0)
    wmat[:, 9, :] = np.repeat(t.T, 3, axis=1)  # [3, CP]
    wmat16 = wmat.astype(np.float16)

    # ---- shard + pad the big tensors ----
    mc_all = NCORES * MC
    npad = mc_all - M_TOTAL
    in_maps = []
    for core in range(NCORES):
        ms, me = core * 12500, (core + 1) * 12500
        n_real = me - ms
        # pole_3ds shard [MC, 9]
        xnat = np.zeros((MC, 9), np.float32)
        xnat[:n_real] = pole_3ds[ms:me].reshape(n_real, 9)
        xt = np.zeros((10, MC), np.float16)
        xt[:9, :n_real] = xnat[:n_real].T.astype(np.float16)
        xt[9, :] = 1.0
        # obs planes [CP, MC], pre-folded by -principal point
        ou = np.zeros((CP, MC), np.float16)
        ov = np.zeros((CP, MC), np.float16)
        ou[:, :n_real] = (pole_2ds[ms:me, :, :, 0].reshape(n_real, CP)
                          - u0_cp[None, :]).T
        ov[:, :n_real] = (pole_2ds[ms:me, :, :, 1].reshape(n_real, CP)
                          - v0_cp[None, :]).T
        mk = np.zeros((CP, MC), np.float16)
        mk[:, :n_real] = np.repeat(
            mask[ms:me].astype(np.float16), 3, axis=1).T
        in_maps.append({
            "obs_u": ou, "obs_v": ov, "maskf": mk,
            "xt": xt, "xn": xnat, "wmat": wmat16, "consts": consts,
        })

    nc = _build_nc(a_coef, b_coef, s)

    from concourse.bass_utils import run_bass_kernel_spmd
    res = run_bass_kernel_spmd(nc, in_maps, core_ids=list(range(NCORES)))
    grand = 0.0
    for r in res.results:
        grand += float(np.asarray(r["out"], np.float64).sum())
    # padded images contribute |0 - s| = s to the len loss each
    loss = W_LOSS * (grand - npad * s) / M_TOTAL
    return np.float32(loss)

